# revision 13
# baseline (speedup 1.0000x reference)
"""Trainium2 Bass kernel for nn_CausalSelfAttention_6442450944521.

Sparse-attention causal self-attention block:
  B=4, T=2048 (rows<512: full attention over cols<512; rows>=512: causal),
  E=1024, H=16, D=64.

Sharding: batch (4) x head-group (2 groups of 8 heads) across 8 cores.
Each core computes, for its (batch b, head-group g):
  qkv^T projections (Q^T,K^T in [D,T] layout; V in natural [T,D] layout),
  block-sparse attention via S^T = K Q^T tiles (softmax denominators come
  free from a ones-column packed next to V), and its row-slice of the
  output projection. The two head-group partials per batch are summed on
  the host (row-parallel tensor parallelism); v-bias and proj-bias are
  folded in exactly on the host.

All matmuls stream as float32r (full PE rate at moving-dim 512).
"""

import sys

if "/opt/trn_rl_repo" not in sys.path:
    sys.path.insert(0, "/opt/trn_rl_repo")

import numpy as np

# Problem constants (hardcoded per harness contract).
B = 4
T = 2048
E = 1024
H = 16
D = 64
NCORES = 8
HPC = H // 2          # heads per core = 8
ESL = HPC * D         # per-core E-slice = 512
P = 128               # SBUF/PSUM partitions
TG = 512              # matmul moving-dim tile (q-group width)
NTG = T // TG         # 4
NTT = T // P          # 16
NEC = E // P          # 8 contraction chunks over E
NPAIR = HPC // 2      # 4 head-pair tiles

_CACHE = {}


def _build_program():
    import concourse.bass as bass
    import concourse.tile as tile
    from concourse import bacc, mybir

    f32 = mybir.dt.float32
    f32r = mybir.dt.float32r
    Exp = mybir.ActivationFunctionType.Exp
    Copy = mybir.ActivationFunctionType.Copy

    nc = bacc.Bacc("TRN2", target_bir_lowering=False, debug=False,
                   num_devices=NCORES)

    xT = nc.dram_tensor("xT", [E, T], f32, kind="ExternalInput").ap()
    wq = nc.dram_tensor("wq", [E, ESL], f32, kind="ExternalInput").ap()
    wk = nc.dram_tensor("wk", [E, ESL], f32, kind="ExternalInput").ap()
    wv = nc.dram_tensor("wv", [E, ESL], f32, kind="ExternalInput").ap()
    wp = nc.dram_tensor("wp", [ESL, E], f32, kind="ExternalInput").ap()
    bq = nc.dram_tensor("bq", [ESL, 1], f32, kind="ExternalInput").ap()
    bk = nc.dram_tensor("bk", [ESL, 1], f32, kind="ExternalInput").ap()
    trimask = nc.dram_tensor("trimask", [P, 3 * TG], f32,
                             kind="ExternalInput").ap()
    out = nc.dram_tensor("out", [T, E], f32, kind="ExternalOutput").ap()

    def r(ap):
        return ap.bitcast(f32r)

    with tile.TileContext(nc) as tc:
        _body(nc, tc, tile, mybir, bass, r, f32, Exp, Copy,
              xT, wq, wk, wv, wp, bq, bk, trimask, out)

    nc.compile()
    return nc


def _body(nc, tc, tile, mybir, bass, r, f32, Exp, Copy,
          xT, wq, wk, wv, wp, bq, bk, trimask, out):
    f32r = mybir.dt.float32r
    # Pools with explicit (non-nested) lifetimes.
    cms = {}

    def open_pool(name, bufs, space=None, side=None):
        kw = {}
        if space:
            kw["space"] = space
        if side:
            kw["side"] = side
        cm = tc.tile_pool(name=name, bufs=bufs, **kw)
        pool = cm.__enter__()
        cms[id(pool)] = cm
        return pool

    def close_pool(pool):
        cms.pop(id(pool)).__exit__(None, None, None)

    singles = open_pool("singles", 1)          # mask + biases, whole program
    yT_pool = open_pool("yTpool", 1)           # y^T, until end of program

    mask_t = singles.tile([P, 3 * TG], f32r, tag="mask", name="mask")
    bias_t = singles.tile([P, 2 * NPAIR], f32, tag="bias", name="bias")
    nc.sync.dma_start(out=mask_t[:], in_=trimask.bitcast(f32r))
    for pt in range(NPAIR):
        nc.sync.dma_start(out=bias_t[:, pt:pt + 1],
                          in_=bq[pt * P:(pt + 1) * P, :])
        nc.sync.dma_start(out=bias_t[:, NPAIR + pt:NPAIR + pt + 1],
                          in_=bk[pt * P:(pt + 1) * P, :])

    yT_t = [yT_pool.tile([P, T], f32r, tag=f"yT{i}", name=f"yT{i}")
            for i in range(NPAIR)]

    # ---------------- Phase 1a: Q^T, K^T = (x @ Wq|k)^T + bias ------------
    # x^T column strips are streamed from DRAM per q-group (x^T is never
    # fully resident).
    qk_pool = open_pool("qkpool", 1, side="right")   # until end of attention
    qT_t = [qk_pool.tile([P, T], f32r, tag=f"qT{i}", name=f"qT{i}")
            for i in range(NPAIR)]
    kT_t = [qk_pool.tile([P, T], f32r, tag=f"kT{i}", name=f"kT{i}")
            for i in range(NPAIR)]
    ps1 = open_pool("ps1", 4, space="PSUM")
    wqk_pool = open_pool("wqk", 2 * NEC * NPAIR)
    xs_pool = open_pool("xs", 2 * NEC)
    wq_c, wk_c = {}, {}
    for pt in range(NPAIR):
        for ec in range(NEC):
            t = wqk_pool.tile([P, P], f32r, tag="wq", name="wqc")
            nc.sync.dma_start(out=t[:], in_=wq[ec * P:(ec + 1) * P,
                                              pt * P:(pt + 1) * P].bitcast(f32r))
            wq_c[(pt, ec)] = t
            t = wqk_pool.tile([P, P], f32r, tag="wk", name="wkc")
            nc.sync.dma_start(out=t[:], in_=wk[ec * P:(ec + 1) * P,
                                              pt * P:(pt + 1) * P].bitcast(f32r))
            wk_c[(pt, ec)] = t
    for tg in range(NTG):
        cs = slice(tg * TG, (tg + 1) * TG)
        xs_c = []
        for ec in range(NEC):
            t = xs_pool.tile([P, TG], f32r, tag="xs", name="xsc")
            nc.sync.dma_start(out=t[:], in_=xT[ec * P:(ec + 1) * P, cs].bitcast(f32r))
            xs_c.append(t)
        for pt in range(NPAIR):
            psq = ps1.tile([P, TG], f32, tag="qk", name="psq")
            for ec in range(NEC):
                nc.tensor.matmul(psq[:], lhsT=wq_c[(pt, ec)][:],
                                 rhs=xs_c[ec][:],
                                 start=(ec == 0), stop=(ec == NEC - 1))
            nc.scalar.add(qT_t[pt][:, cs], psq[:], bias_t[:, pt:pt + 1])
            psk = ps1.tile([P, TG], f32, tag="qk", name="psk")
            for ec in range(NEC):
                nc.tensor.matmul(psk[:], lhsT=wk_c[(pt, ec)][:],
                                 rhs=xs_c[ec][:],
                                 start=(ec == 0), stop=(ec == NEC - 1))
            nc.scalar.add(kT_t[pt][:, cs], psk[:],
                          bias_t[:, NPAIR + pt:NPAIR + pt + 1])
    close_pool(xs_pool)
    close_pool(wqk_pool)

    # ---------------- Phase 1b: V = x @ Wv (natural layout + ones col) ----
    v_pool = open_pool("vpool", 1, side="right")     # until end of attention
    v_t = [v_pool.tile([P, HPC, D + 1], f32r, tag=f"v{i}", name=f"v{i}")
           for i in range(NTT)]
    wv_pool = open_pool("wv", NEC)
    xv_pool = open_pool("xv", 2 * NEC)
    wv_c = []
    for ec in range(NEC):
        t = wv_pool.tile([P, ESL], f32r, tag="wv", name="wvc")
        nc.sync.dma_start(out=t[:], in_=wv[ec * P:(ec + 1) * P, :].bitcast(f32r))
        wv_c.append(t)
    for tt in range(NTT):
        ts_ = slice(tt * P, (tt + 1) * P)
        xv_c = []
        for ec in range(NEC):
            t = xv_pool.tile([P, P], f32r, tag="xv", name="xvc")
            nc.sync.dma_start(out=t[:], in_=xT[ec * P:(ec + 1) * P, ts_].bitcast(f32r))
            xv_c.append(t)
        psv = ps1.tile([P, HPC, D], f32, tag="qk", name="psv")
        for ec in range(NEC):
            nc.tensor.matmul(psv[:, :, :], lhsT=xv_c[ec][:],
                             rhs=wv_c[ec][:],
                             start=(ec == 0), stop=(ec == NEC - 1))
        nc.vector.memset(v_t[tt][:, :, D:D + 1].bitcast(f32), 1.0)
        nc.scalar.activation(v_t[tt][:, :, 0:D], psv[:, :, :], Copy)
    close_pool(xv_pool)
    close_pool(wv_pool)
    close_pool(ps1)

    # ---------------- Phase 2: block-sparse attention ---------------------
    ps_s = open_pool("ps_s", 3, space="PSUM")
    ps_o = open_pool("ps_o", 5, space="PSUM")
    pT_pool = open_pool("pT", 4)
    dn_pool = open_pool("dn", 6)
    rc_pool = open_pool("rc", 2)
    bc_pool = open_pool("bc", 2)
    on_pool = open_pool("on", 2)
    dr_pool = open_pool("dr", 2, space="DRAM")
    for h in range(HPC):
        pt, hr = divmod(h, 2)
        rb = hr * 64
        po_list = []
        dst_list = []
        for qg in range(NTG):
            qb = qg * TG
            qs = slice(qb, qb + TG)
            if qg == 0:
                kts = [(kt, None) for kt in range(4)]
            else:
                kts = ([(kt, None) for kt in range(qb // P)] +
                       [(qb // P + m, -m * P) for m in range(4)])
            po = ps_o.tile([D + 1, TG], f32, tag="o", name="po")
            po_list.append(po)
            nkt = len(kts)
            for i, (kt, off) in enumerate(kts):
                ks = slice(kt * P, (kt + 1) * P)
                pss = ps_s.tile([P, TG], f32, tag="s", name="pss")
                nc.tensor.matmul(pss[:], lhsT=kT_t[pt][rb:rb + 64, ks],
                                 rhs=qT_t[pt][rb:rb + 64, qs],
                                 start=True, stop=True)
                pT = pT_pool.tile([P, TG], f32r, tag="pT", name="pT")
                nc.scalar.activation(pT[:], pss[:], Exp, scale=0.125)
                if off is not None:
                    c0 = off + TG
                    nc.vector.tensor_mul(pT[:], pT[:], mask_t[:, c0:c0 + TG])
                nc.tensor.matmul(po[:], lhsT=v_t[kt][:, h, :], rhs=pT[:],
                                 start=(i == 0), stop=(i == nkt - 1))
            # Stash the denominator row (lives at partition 64 of po).
            dst = dn_pool.tile([D + 1, TG], f32, tag="dst", name="dst")
            dst_list.append(dst)
            nc.scalar.activation(dst[D:D + 1, :], po[D:D + 1, :], Copy)
        # Reciprocal of all 4 denominator rows at once (partitions 0..3).
        dd = rc_pool.tile([NTG, TG], f32, tag="ddst", name="dd")
        for qg in range(NTG):
            nc.sync.dma_start(out=dd[qg:qg + 1, :],
                              in_=dst_list[qg][D:D + 1, :])
        rec = rc_pool.tile([NTG, TG], f32, tag="rec", name="rec")
        scr = rc_pool.tile([NTG, TG], f32, tag="scr", name="scr")
        nc.vector.reciprocal_approx_accurate(out=rec[:], in_=dd[:],
                                             scratch=scr[:])
        rec_d = dr_pool.tile([NTG, TG], f32, tag="rec_d", name="rec_d")
        nc.sync.dma_start(out=rec_d[:], in_=rec[:])
        for qg in range(NTG):
            row = rec_d[qg:qg + 1, :]
            bcast_in = bass.AP(tensor=row.tensor, offset=row.offset,
                               ap=[[0, D]] + [list(a) for a in row.ap[1:]])
            bc = bc_pool.tile([D, TG], f32, tag="bc", name="bc")
            nc.sync.dma_start(out=bc[:], in_=bcast_in)
            on = on_pool.tile([D, TG], f32r, tag="on", name="on")
            nc.vector.tensor_mul(on[:], po_list[qg][0:D, :], bc[:])
            nc.sync.dma_start(out=yT_t[pt][rb:rb + 64, qg * TG:(qg + 1) * TG],
                              in_=on[:])
    close_pool(dr_pool)
    close_pool(on_pool)
    close_pool(bc_pool)
    close_pool(rc_pool)
    close_pool(dn_pool)
    close_pool(pT_pool)
    close_pool(ps_o)
    close_pool(ps_s)
    close_pool(v_pool)
    close_pool(qk_pool)

    # ---------------- Phase 3: out = y @ Wp (row-parallel partial) --------
    ps_p = open_pool("ps_p", 3, space="PSUM")
    wp_pool = open_pool("wp", 2 * NPAIR)
    ot_pool = open_pool("ot", 4)
    wp_t = {}
    for c in range(NPAIR):
        for ng in range(E // TG):
            t = wp_pool.tile([P, TG], f32r, tag="wp", name="wpc")
            nc.sync.dma_start(out=t[:], in_=wp[c * P:(c + 1) * P,
                                              ng * TG:(ng + 1) * TG].bitcast(f32r))
            wp_t[(c, ng)] = t
    for tt in range(NTT):
        ts_ = slice(tt * P, (tt + 1) * P)
        for ng in range(E // TG):
            pp = ps_p.tile([P, TG], f32, tag="proj", name="pp")
            for c in range(NPAIR):
                nc.tensor.matmul(pp[:], lhsT=yT_t[c][:, ts_],
                                 rhs=wp_t[(c, ng)][:],
                                 start=(c == 0), stop=(c == NPAIR - 1))
            ot = ot_pool.tile([P, TG], f32, tag="ot", name="ot")
            nc.vector.tensor_copy(ot[:], pp[:])
            nc.sync.dma_start(out=out[ts_, ng * TG:(ng + 1) * TG], in_=ot[:])
    close_pool(ot_pool)
    close_pool(wp_pool)
    close_pool(ps_p)
    close_pool(yT_pool)
    close_pool(singles)


def _get_program():
    if "nc" not in _CACHE:
        _CACHE["nc"] = _build_program()
    return _CACHE["nc"]


def make_in_maps(x, W_qkv, b_qkv, W_proj):
    """Per-core input dicts: core c -> (batch c%4, head-group c//4)."""
    x = np.asarray(x, np.float32)
    W_qkv = np.asarray(W_qkv, np.float32)
    b_qkv = np.asarray(b_qkv, np.float32)
    tri = ((np.arange(3 * TG)[None, :] - TG) >=
           np.arange(P)[:, None]).astype(np.float32)
    in_maps = []
    for c in range(NCORES):
        b, g = c % B, c // B
        gs = slice(g * ESL, (g + 1) * ESL)
        in_maps.append({
            "xT": np.ascontiguousarray(x[b].T),
            "wq": np.ascontiguousarray(W_qkv[:, 0 * E:1 * E][:, gs]),
            "wk": np.ascontiguousarray(W_qkv[:, 1 * E:2 * E][:, gs]),
            "wv": np.ascontiguousarray(W_qkv[:, 2 * E:3 * E][:, gs]),
            "wp": np.ascontiguousarray(np.asarray(W_proj, np.float32)[gs, :]),
            "bq": np.ascontiguousarray(b_qkv[0 * E:1 * E][gs, None]),
            "bk": np.ascontiguousarray(b_qkv[1 * E:2 * E][gs, None]),
            "trimask": tri,
        })
    return in_maps


def gather_output(results, b_qkv, b_proj, W_proj):
    """Sum the two row-parallel partials per batch; fold v/proj biases."""
    b_qkv = np.asarray(b_qkv, np.float64)
    W_proj = np.asarray(W_proj, np.float64)
    b_v = b_qkv[2 * E:3 * E]
    const = b_v @ W_proj + np.asarray(b_proj, np.float64)
    out = np.empty((B, T, E), np.float32)
    for b in range(B):
        out[b] = (results[b]["out"].astype(np.float64) +
                  results[b + B]["out"].astype(np.float64) +
                  const).astype(np.float32)
    return out


def run_on_hw(inputs, trace=False, **kwargs):
    from concourse.bass_utils import run_bass_kernel_spmd
    nc = _get_program()
    in_maps = make_in_maps(inputs["x"], inputs["W_qkv"], inputs["b_qkv"],
                           inputs["W_proj"])
    res = run_bass_kernel_spmd(nc, in_maps, list(range(NCORES)), trace=trace,
                               **kwargs)
    out = gather_output(res.results, inputs["b_qkv"], inputs["b_proj"],
                        inputs["W_proj"])
    return out, res


def kernel(x, W_qkv, b_qkv, W_proj, b_proj):
    out, _ = run_on_hw({"x": x, "W_qkv": W_qkv, "b_qkv": b_qkv,
                        "W_proj": W_proj, "b_proj": b_proj})
    return out


# revision 17
# speedup vs baseline: 1.0657x; 1.0657x over previous
"""Trainium2 Bass kernel for nn_CausalSelfAttention_6442450944521.

Sparse-attention causal self-attention block:
  B=4, T=2048 (rows<512: full attention over cols<512; rows>=512: causal),
  E=1024, H=16, D=64.

Sharding: batch (4) x head-group (2 groups of 8 heads) across 8 cores.
Each core computes, for its (batch b, head-group g):
  qkv^T projections (Q^T,K^T in [D,T] layout; V in natural [T,D] layout),
  block-sparse attention via S^T = K Q^T tiles (softmax denominators come
  free from a ones-column packed next to V), and its row-slice of the
  output projection. The two head-group partials per batch are summed on
  the host (row-parallel tensor parallelism); v-bias and proj-bias are
  folded in exactly on the host.

All matmuls stream as float32r (full PE rate at moving-dim 512).
"""

import sys

if "/opt/trn_rl_repo" not in sys.path:
    sys.path.insert(0, "/opt/trn_rl_repo")

import numpy as np

# Problem constants (hardcoded per harness contract).
B = 4
T = 2048
E = 1024
H = 16
D = 64
NCORES = 8
HPC = H // 2          # heads per core = 8
ESL = HPC * D         # per-core E-slice = 512
P = 128               # SBUF/PSUM partitions
TG = 512              # matmul moving-dim tile (q-group width)
NTG = T // TG         # 4
NTT = T // P          # 16
NEC = E // P          # 8 contraction chunks over E
NPAIR = HPC // 2      # 4 head-pair tiles

_CACHE = {}


def _build_program():
    import concourse.bass as bass
    import concourse.tile as tile
    from concourse import bacc, mybir

    f32 = mybir.dt.float32
    f32r = mybir.dt.float32r
    Exp = mybir.ActivationFunctionType.Exp
    Copy = mybir.ActivationFunctionType.Copy

    nc = bacc.Bacc("TRN2", target_bir_lowering=False, debug=False,
                   num_devices=NCORES)

    xT = nc.dram_tensor("xT", [E, T], f32, kind="ExternalInput").ap()
    wq = nc.dram_tensor("wq", [E, ESL], f32, kind="ExternalInput").ap()
    wk = nc.dram_tensor("wk", [E, ESL], f32, kind="ExternalInput").ap()
    wv = nc.dram_tensor("wv", [E, ESL], f32, kind="ExternalInput").ap()
    wp = nc.dram_tensor("wp", [ESL, E], f32, kind="ExternalInput").ap()
    bq = nc.dram_tensor("bq", [ESL, 1], f32, kind="ExternalInput").ap()
    bk = nc.dram_tensor("bk", [ESL, 1], f32, kind="ExternalInput").ap()
    trimask = nc.dram_tensor("trimask", [P, 3 * TG], f32,
                             kind="ExternalInput").ap()
    out = nc.dram_tensor("out", [T, E], f32, kind="ExternalOutput").ap()

    def r(ap):
        return ap.bitcast(f32r)

    with tile.TileContext(nc) as tc:
        _body(nc, tc, tile, mybir, bass, r, f32, Exp, Copy,
              xT, wq, wk, wv, wp, bq, bk, trimask, out)

    nc.compile()
    return nc


def _body(nc, tc, tile, mybir, bass, r, f32, Exp, Copy,
          xT, wq, wk, wv, wp, bq, bk, trimask, out):
    f32r = mybir.dt.float32r
    # Pools with explicit (non-nested) lifetimes.
    cms = {}

    def open_pool(name, bufs, space=None, side=None):
        kw = {}
        if space:
            kw["space"] = space
        if side:
            kw["side"] = side
        cm = tc.tile_pool(name=name, bufs=bufs, **kw)
        pool = cm.__enter__()
        cms[id(pool)] = cm
        return pool

    def close_pool(pool):
        cms.pop(id(pool)).__exit__(None, None, None)

    singles = open_pool("singles", 1)          # mask + biases, whole program
    yT_pool = open_pool("yTpool", 1)           # y^T, until end of program

    mask_t = singles.tile([P, 3 * TG], f32r, tag="mask", name="mask")
    bias_t = singles.tile([P, 2 * NPAIR], f32, tag="bias", name="bias")
    nc.sync.dma_start(out=mask_t[:], in_=trimask.bitcast(f32r))
    for pt in range(NPAIR):
        nc.sync.dma_start(out=bias_t[:, pt:pt + 1],
                          in_=bq[pt * P:(pt + 1) * P, :])
        nc.sync.dma_start(out=bias_t[:, NPAIR + pt:NPAIR + pt + 1],
                          in_=bk[pt * P:(pt + 1) * P, :])

    yT_t = [yT_pool.tile([P, T], f32r, tag=f"yT{i}", name=f"yT{i}")
            for i in range(NPAIR)]

    # ---------------- Phase 1a: Q^T, K^T = (x @ Wq|k)^T + bias ------------
    # x^T column strips are streamed from DRAM per q-group (x^T is never
    # fully resident).
    qk_pool = open_pool("qkpool", 1, side="right")   # until end of attention
    qT_t = [qk_pool.tile([P, T], f32r, tag=f"qT{i}", name=f"qT{i}")
            for i in range(NPAIR)]
    kT_t = [qk_pool.tile([P, T], f32r, tag=f"kT{i}", name=f"kT{i}")
            for i in range(NPAIR)]
    ps1 = open_pool("ps1", 4, space="PSUM")
    xs_pool = open_pool("xs", 2 * NEC)   # shared by QK strips and V strips
    wqk_pool = open_pool("wqk", 2 * NEC * NPAIR)
    wq_c, wk_c = {}, {}

    def load_wqk(pt):
        for ec in range(NEC):
            t = wqk_pool.tile([P, P], f32r, tag="wq", name="wqc")
            nc.sync.dma_start(out=t[:], in_=wq[ec * P:(ec + 1) * P,
                                              pt * P:(pt + 1) * P].bitcast(f32r))
            wq_c[(pt, ec)] = t
            t = wqk_pool.tile([P, P], f32r, tag="wk", name="wkc")
            nc.sync.dma_start(out=t[:], in_=wk[ec * P:(ec + 1) * P,
                                              pt * P:(pt + 1) * P].bitcast(f32r))
            wk_c[(pt, ec)] = t

    for tg in range(NTG):
        cs = slice(tg * TG, (tg + 1) * TG)
        xs_c = []
        for ec in range(NEC):
            t = xs_pool.tile([P, TG], f32r, tag="xs", name="xsc")
            nc.sync.dma_start(out=t[:], in_=xT[ec * P:(ec + 1) * P, cs].bitcast(f32r))
            xs_c.append(t)
        for pt in range(NPAIR):
            if tg == 0:
                load_wqk(pt)   # consumption order: overlap first matmuls
            psq = ps1.tile([P, TG], f32, tag="qk", name="psq")
            for ec in range(NEC):
                nc.tensor.matmul(psq[:], lhsT=wq_c[(pt, ec)][:],
                                 rhs=xs_c[ec][:],
                                 start=(ec == 0), stop=(ec == NEC - 1))
            nc.scalar.add(qT_t[pt][:, cs], psq[:], bias_t[:, pt:pt + 1])
            psk = ps1.tile([P, TG], f32, tag="qk", name="psk")
            for ec in range(NEC):
                nc.tensor.matmul(psk[:], lhsT=wk_c[(pt, ec)][:],
                                 rhs=xs_c[ec][:],
                                 start=(ec == 0), stop=(ec == NEC - 1))
            nc.scalar.add(kT_t[pt][:, cs], psk[:],
                          bias_t[:, NPAIR + pt:NPAIR + pt + 1])
    close_pool(wqk_pool)

    # ---------------- Phase 1b: V = x @ Wv (natural layout + ones col) ----
    v_pool = open_pool("vpool", 1, side="right")     # until end of attention
    v_t = [v_pool.tile([P, HPC, D + 1], f32r, tag=f"v{i}", name=f"v{i}")
           for i in range(NTT)]
    wv_pool = open_pool("wv", NEC)
    wv_c = []
    for ec in range(NEC):
        t = wv_pool.tile([P, ESL], f32r, tag="wv", name="wvc")
        nc.sync.dma_start(out=t[:], in_=wv[ec * P:(ec + 1) * P, :].bitcast(f32r))
        wv_c.append(t)
    for tt in range(NTT):
        ts_ = slice(tt * P, (tt + 1) * P)
        xv_c = []
        for ec in range(NEC):
            t = xs_pool.tile([P, P], f32r, tag="xv", name="xvc")
            nc.sync.dma_start(out=t[:], in_=xT[ec * P:(ec + 1) * P, ts_].bitcast(f32r))
            xv_c.append(t)
        psv = ps1.tile([P, HPC, D], f32, tag="qk", name="psv")
        for ec in range(NEC):
            nc.tensor.matmul(psv[:, :, :], lhsT=xv_c[ec][:],
                             rhs=wv_c[ec][:],
                             start=(ec == 0), stop=(ec == NEC - 1))
        nc.vector.memset(v_t[tt][:, :, D:D + 1].bitcast(f32), 1.0)
        nc.scalar.activation(v_t[tt][:, :, 0:D], psv[:, :, :], Copy)
    close_pool(wv_pool)
    close_pool(xs_pool)
    close_pool(ps1)

    # ---------------- Phase 2: block-sparse attention ---------------------
    ps_s = open_pool("ps_s", 4, space="PSUM")
    ps_o = open_pool("ps_o", 4, space="PSUM")
    pT_pool = open_pool("pT", 6)
    ob_pool = open_pool("ob", 4)
    rc_pool = open_pool("rc", 2)
    bc_pool = open_pool("bc", 2)
    on_pool = open_pool("on", 2)
    dr_pool = open_pool("dr", 2, space="DRAM")
    for h in range(HPC):
        pt, hr = divmod(h, 2)
        rb = hr * 64
        ob_list = []
        for qg in range(NTG):
            qb = qg * TG
            qs = slice(qb, qb + TG)
            if qg == 0:
                kts = [(kt, None) for kt in range(4)]
            else:
                kts = ([(kt, None) for kt in range(qb // P)] +
                       [(qb // P + m, -m * P) for m in range(4)])
            po = ps_o.tile([D + 1, TG], f32, tag="o", name="po")
            nkt = len(kts)
            for i, (kt, off) in enumerate(kts):
                ks = slice(kt * P, (kt + 1) * P)
                pss = ps_s.tile([P, TG], f32, tag="s", name="pss")
                nc.tensor.matmul(pss[:], lhsT=kT_t[pt][rb:rb + 64, ks],
                                 rhs=qT_t[pt][rb:rb + 64, qs],
                                 start=True, stop=True)
                pT = pT_pool.tile([P, TG], f32r, tag="pT", name="pT")
                nc.scalar.activation(pT[:], pss[:], Exp, scale=0.125)
                if off is not None:
                    c0 = off + TG
                    nc.vector.tensor_mul(pT[:], pT[:], mask_t[:, c0:c0 + TG])
                nc.tensor.matmul(po[:], lhsT=v_t[kt][:, h, :], rhs=pT[:],
                                 start=(i == 0), stop=(i == nkt - 1))
            # Evacuate the whole accumulator at once (frees the PSUM bank
            # immediately so the next head's O matmuls never stall).
            ob = ob_pool.tile([D + 1, TG], f32, tag="ob", name="ob")
            ob_list.append(ob)
            nc.vector.tensor_copy(ob[:], po[:])
        # Reciprocal of all 4 denominator rows at once (partitions 0..3).
        dd = rc_pool.tile([NTG, TG], f32, tag="ddst", name="dd")
        for qg in range(NTG):
            nc.sync.dma_start(out=dd[qg:qg + 1, :],
                              in_=ob_list[qg][D:D + 1, :])
        rec = rc_pool.tile([NTG, TG], f32, tag="rec", name="rec")
        scr = rc_pool.tile([NTG, TG], f32, tag="scr", name="scr")
        nc.vector.reciprocal_approx_accurate(out=rec[:], in_=dd[:],
                                             scratch=scr[:])
        rec_d = dr_pool.tile([NTG, TG], f32, tag="rec_d", name="rec_d")
        nc.sync.dma_start(out=rec_d[:], in_=rec[:])
        for qg in range(NTG):
            row = rec_d[qg:qg + 1, :]
            bcast_in = bass.AP(tensor=row.tensor, offset=row.offset,
                               ap=[[0, D]] + [list(a) for a in row.ap[1:]])
            bc = bc_pool.tile([D, TG], f32, tag="bc", name="bc")
            nc.sync.dma_start(out=bc[:], in_=bcast_in)
            on = on_pool.tile([D, TG], f32r, tag="on", name="on")
            nc.vector.tensor_mul(on[:], ob_list[qg][0:D, :], bc[:])
            nc.sync.dma_start(out=yT_t[pt][rb:rb + 64, qg * TG:(qg + 1) * TG],
                              in_=on[:])
    close_pool(dr_pool)
    close_pool(on_pool)
    close_pool(bc_pool)
    close_pool(rc_pool)
    close_pool(ob_pool)
    close_pool(pT_pool)
    close_pool(ps_o)
    close_pool(ps_s)
    close_pool(v_pool)
    close_pool(qk_pool)

    # ---------------- Phase 3: out = y @ Wp (row-parallel partial) --------
    ps_p = open_pool("ps_p", 3, space="PSUM")
    wp_pool = open_pool("wp", 2 * NPAIR)
    ot_pool = open_pool("ot", 4)
    wp_t = {}
    for c in range(NPAIR):
        for ng in range(E // TG):
            t = wp_pool.tile([P, TG], f32r, tag="wp", name="wpc")
            nc.sync.dma_start(out=t[:], in_=wp[c * P:(c + 1) * P,
                                              ng * TG:(ng + 1) * TG].bitcast(f32r))
            wp_t[(c, ng)] = t
    for tt in range(NTT):
        ts_ = slice(tt * P, (tt + 1) * P)
        for ng in range(E // TG):
            pp = ps_p.tile([P, TG], f32, tag="proj", name="pp")
            for c in range(NPAIR):
                nc.tensor.matmul(pp[:], lhsT=yT_t[c][:, ts_],
                                 rhs=wp_t[(c, ng)][:],
                                 start=(c == 0), stop=(c == NPAIR - 1))
            ot = ot_pool.tile([P, TG], f32, tag="ot", name="ot")
            nc.vector.tensor_copy(ot[:], pp[:])
            nc.sync.dma_start(out=out[ts_, ng * TG:(ng + 1) * TG], in_=ot[:])
    close_pool(ot_pool)
    close_pool(wp_pool)
    close_pool(ps_p)
    close_pool(yT_pool)
    close_pool(singles)


def _get_program():
    if "nc" not in _CACHE:
        _CACHE["nc"] = _build_program()
    return _CACHE["nc"]


def make_in_maps(x, W_qkv, b_qkv, W_proj):
    """Per-core input dicts: core c -> (batch c%4, head-group c//4)."""
    x = np.asarray(x, np.float32)
    W_qkv = np.asarray(W_qkv, np.float32)
    b_qkv = np.asarray(b_qkv, np.float32)
    tri = ((np.arange(3 * TG)[None, :] - TG) >=
           np.arange(P)[:, None]).astype(np.float32)
    in_maps = []
    for c in range(NCORES):
        b, g = c % B, c // B
        gs = slice(g * ESL, (g + 1) * ESL)
        in_maps.append({
            "xT": np.ascontiguousarray(x[b].T),
            "wq": np.ascontiguousarray(W_qkv[:, 0 * E:1 * E][:, gs]),
            "wk": np.ascontiguousarray(W_qkv[:, 1 * E:2 * E][:, gs]),
            "wv": np.ascontiguousarray(W_qkv[:, 2 * E:3 * E][:, gs]),
            "wp": np.ascontiguousarray(np.asarray(W_proj, np.float32)[gs, :]),
            "bq": np.ascontiguousarray(b_qkv[0 * E:1 * E][gs, None]),
            "bk": np.ascontiguousarray(b_qkv[1 * E:2 * E][gs, None]),
            "trimask": tri,
        })
    return in_maps


def gather_output(results, b_qkv, b_proj, W_proj):
    """Sum the two row-parallel partials per batch; fold v/proj biases."""
    b_qkv = np.asarray(b_qkv, np.float64)
    W_proj = np.asarray(W_proj, np.float64)
    b_v = b_qkv[2 * E:3 * E]
    const = b_v @ W_proj + np.asarray(b_proj, np.float64)
    out = np.empty((B, T, E), np.float32)
    for b in range(B):
        out[b] = (results[b]["out"].astype(np.float64) +
                  results[b + B]["out"].astype(np.float64) +
                  const).astype(np.float32)
    return out


def run_on_hw(inputs, trace=False, **kwargs):
    from concourse.bass_utils import run_bass_kernel_spmd
    nc = _get_program()
    in_maps = make_in_maps(inputs["x"], inputs["W_qkv"], inputs["b_qkv"],
                           inputs["W_proj"])
    res = run_bass_kernel_spmd(nc, in_maps, list(range(NCORES)), trace=trace,
                               **kwargs)
    out = gather_output(res.results, inputs["b_qkv"], inputs["b_proj"],
                        inputs["W_proj"])
    return out, res


def kernel(x, W_qkv, b_qkv, W_proj, b_proj):
    out, _ = run_on_hw({"x": x, "W_qkv": W_qkv, "b_qkv": b_qkv,
                        "W_proj": W_proj, "b_proj": b_proj})
    return out


# revision 18
# speedup vs baseline: 1.1849x; 1.1119x over previous
"""Trainium2 Bass kernel for nn_CausalSelfAttention_6442450944521.

Sparse-attention causal self-attention block:
  B=4, T=2048 (rows<512: full attention over cols<512; rows>=512: causal),
  E=1024, H=16, D=64.

Sharding: batch (4) x head-group (2 groups of 8 heads) across 8 cores.
Each core computes, for its (batch b, head-group g):
  qkv^T projections (Q^T,K^T in [D,T] layout; V in natural [T,D] layout),
  block-sparse attention via S^T = K Q^T tiles (softmax denominators come
  free from a ones-column packed next to V), and its row-slice of the
  output projection. The two head-group partials per batch are summed on
  the host (row-parallel tensor parallelism); v-bias and proj-bias are
  folded in exactly on the host.

Matmul inputs stream as bf16 (1 cyc/row on the PE) by default;
KMODE=f32r switches to float32r (2 cyc/row, ~20x lower error).
Accumulation is always fp32 in PSUM; softmax denominators and the
normalization are fp32.
"""

import os
import sys

if "/opt/trn_rl_repo" not in sys.path:
    sys.path.insert(0, "/opt/trn_rl_repo")

import numpy as np

# Problem constants (hardcoded per harness contract).
B = 4
T = 2048
E = 1024
H = 16
D = 64
NCORES = 8
HPC = H // 2          # heads per core = 8
ESL = HPC * D         # per-core E-slice = 512
P = 128               # SBUF/PSUM partitions
TG = 512              # matmul moving-dim tile (q-group width)
NTG = T // TG         # 4
NTT = T // P          # 16
NEC = E // P          # 8 contraction chunks over E
NPAIR = HPC // 2      # 4 head-pair tiles

MODE = os.environ.get("KMODE", "bf16")   # "bf16" | "f32r"

_CACHE = {}


def _build_program(mode):
    import concourse.bass as bass
    import concourse.tile as tile
    from concourse import bacc, mybir

    f32 = mybir.dt.float32
    idt = mybir.dt.bfloat16 if mode == "bf16" else mybir.dt.float32
    # dtype of DRAM inputs holding matmul operands (host converts for bf16)

    nc = bacc.Bacc("TRN2", target_bir_lowering=False, debug=False,
                   num_devices=NCORES)

    xT = nc.dram_tensor("xT", [E, T], idt, kind="ExternalInput").ap()
    wq = nc.dram_tensor("wq", [E, ESL], idt, kind="ExternalInput").ap()
    wk = nc.dram_tensor("wk", [E, ESL], idt, kind="ExternalInput").ap()
    wv = nc.dram_tensor("wv", [E, ESL], idt, kind="ExternalInput").ap()
    wp = nc.dram_tensor("wp", [ESL, E], idt, kind="ExternalInput").ap()
    bq = nc.dram_tensor("bq", [ESL, 1], f32, kind="ExternalInput").ap()
    bk = nc.dram_tensor("bk", [ESL, 1], f32, kind="ExternalInput").ap()
    trimask = nc.dram_tensor("trimask", [P, 3 * TG], idt,
                             kind="ExternalInput").ap()
    out = nc.dram_tensor("out", [T, E], f32, kind="ExternalOutput").ap()

    with tile.TileContext(nc) as tc:
        _body(nc, tc, tile, mybir, bass, mode,
              xT, wq, wk, wv, wp, bq, bk, trimask, out)

    nc.compile()
    return nc


def _body(nc, tc, tile, mybir, bass, mode,
          xT, wq, wk, wv, wp, bq, bk, trimask, out):
    f32 = mybir.dt.float32
    f32r = mybir.dt.float32r
    Exp = mybir.ActivationFunctionType.Exp
    Copy = mybir.ActivationFunctionType.Copy
    bf16_mode = mode == "bf16"
    # SBUF dtype for matmul operands; in f32r mode tiles are declared
    # float32r (walrus requires f32r-producing instructions) and DRAM
    # sources are bitcast.
    sdt = mybir.dt.bfloat16 if bf16_mode else f32r

    def src(ap):
        return ap if bf16_mode else ap.bitcast(f32r)

    cms = {}

    def open_pool(name, bufs, space=None, side=None):
        kw = {}
        if space:
            kw["space"] = space
        if side:
            kw["side"] = side
        cm = tc.tile_pool(name=name, bufs=bufs, **kw)
        pool = cm.__enter__()
        cms[id(pool)] = cm
        return pool

    def close_pool(pool):
        cms.pop(id(pool)).__exit__(None, None, None)

    def evac_bias(out_ap, psum_ap, bias_ap):
        if bf16_mode:
            nc.vector.tensor_scalar_add(out_ap, psum_ap, bias_ap)
        else:
            nc.scalar.add(out_ap, psum_ap, bias_ap)

    def evac_copy(out_ap, psum_ap):
        if bf16_mode:
            nc.vector.tensor_copy(out_ap, psum_ap)
        else:
            nc.scalar.activation(out_ap, psum_ap, Copy)

    singles = open_pool("singles", 1)          # mask + biases, whole program
    yT_pool = open_pool("yTpool", 1)           # y^T, until end of program

    mask_t = singles.tile([P, 3 * TG], sdt, tag="mask", name="mask")
    bias_t = singles.tile([P, 2 * NPAIR], f32, tag="bias", name="bias")
    nc.sync.dma_start(out=mask_t[:], in_=src(trimask))
    for pt in range(NPAIR):
        nc.sync.dma_start(out=bias_t[:, pt:pt + 1],
                          in_=bq[pt * P:(pt + 1) * P, :])
        nc.sync.dma_start(out=bias_t[:, NPAIR + pt:NPAIR + pt + 1],
                          in_=bk[pt * P:(pt + 1) * P, :])

    yT_t = [yT_pool.tile([P, T], sdt, tag=f"yT{i}", name=f"yT{i}")
            for i in range(NPAIR)]

    # ---------------- Phase 1a: Q^T, K^T = (x @ Wq|k)^T + bias ------------
    qk_pool = open_pool("qkpool", 1, side="right")   # until end of attention
    qT_t = [qk_pool.tile([P, T], sdt, tag=f"qT{i}", name=f"qT{i}")
            for i in range(NPAIR)]
    kT_t = [qk_pool.tile([P, T], sdt, tag=f"kT{i}", name=f"kT{i}")
            for i in range(NPAIR)]
    ps1 = open_pool("ps1", 4, space="PSUM")
    xs_pool = open_pool("xs", 2 * NEC)   # shared by QK strips and V strips
    wqk_pool = open_pool("wqk", 2 * NEC * NPAIR)
    wq_c, wk_c = {}, {}

    def load_wqk(pt):
        for ec in range(NEC):
            t = wqk_pool.tile([P, P], sdt, tag="wq", name="wqc")
            nc.sync.dma_start(out=t[:], in_=src(wq[ec * P:(ec + 1) * P,
                                                   pt * P:(pt + 1) * P]))
            wq_c[(pt, ec)] = t
            t = wqk_pool.tile([P, P], sdt, tag="wk", name="wkc")
            nc.sync.dma_start(out=t[:], in_=src(wk[ec * P:(ec + 1) * P,
                                                   pt * P:(pt + 1) * P]))
            wk_c[(pt, ec)] = t

    for tg in range(NTG):
        cs = slice(tg * TG, (tg + 1) * TG)
        xs_c = []
        for ec in range(NEC):
            t = xs_pool.tile([P, TG], sdt, tag="xs", name="xsc")
            nc.sync.dma_start(out=t[:], in_=src(xT[ec * P:(ec + 1) * P, cs]))
            xs_c.append(t)
        for pt in range(NPAIR):
            if tg == 0:
                load_wqk(pt)   # consumption order: overlap first matmuls
            psq = ps1.tile([P, TG], f32, tag="qk", name="psq")
            for ec in range(NEC):
                nc.tensor.matmul(psq[:], lhsT=wq_c[(pt, ec)][:],
                                 rhs=xs_c[ec][:],
                                 start=(ec == 0), stop=(ec == NEC - 1))
            evac_bias(qT_t[pt][:, cs], psq[:], bias_t[:, pt:pt + 1])
            psk = ps1.tile([P, TG], f32, tag="qk", name="psk")
            for ec in range(NEC):
                nc.tensor.matmul(psk[:], lhsT=wk_c[(pt, ec)][:],
                                 rhs=xs_c[ec][:],
                                 start=(ec == 0), stop=(ec == NEC - 1))
            evac_bias(kT_t[pt][:, cs], psk[:],
                      bias_t[:, NPAIR + pt:NPAIR + pt + 1])
    close_pool(wqk_pool)

    # ---------------- Phase 1b: V = x @ Wv (natural layout + ones col) ----
    v_pool = open_pool("vpool", 1, side="right")     # until end of attention
    v_t = [v_pool.tile([P, HPC, D + 1], sdt, tag=f"v{i}", name=f"v{i}")
           for i in range(NTT)]
    wv_pool = open_pool("wv", NEC)
    wv_c = []
    for ec in range(NEC):
        t = wv_pool.tile([P, ESL], sdt, tag="wv", name="wvc")
        nc.sync.dma_start(out=t[:], in_=src(wv[ec * P:(ec + 1) * P, :]))
        wv_c.append(t)
    for tt in range(NTT):
        ts_ = slice(tt * P, (tt + 1) * P)
        xv_c = []
        for ec in range(NEC):
            t = xs_pool.tile([P, P], sdt, tag="xv", name="xvc")
            nc.sync.dma_start(out=t[:], in_=src(xT[ec * P:(ec + 1) * P, ts_]))
            xv_c.append(t)
        psv = ps1.tile([P, HPC, D], f32, tag="qk", name="psv")
        for ec in range(NEC):
            nc.tensor.matmul(psv[:, :, :], lhsT=xv_c[ec][:],
                             rhs=wv_c[ec][:],
                             start=(ec == 0), stop=(ec == NEC - 1))
        ones = v_t[tt][:, :, D:D + 1]
        nc.vector.memset(ones if bf16_mode else ones.bitcast(f32), 1.0)
        evac_copy(v_t[tt][:, :, 0:D], psv[:, :, :])
    close_pool(wv_pool)
    close_pool(xs_pool)
    close_pool(ps1)

    # ---------------- Phase 2: block-sparse attention ---------------------
    ps_s = open_pool("ps_s", 4, space="PSUM")
    ps_o = open_pool("ps_o", 4, space="PSUM")
    pT_pool = open_pool("pT", 6)
    ob_pool = open_pool("ob", 4)
    rc_pool = open_pool("rc", 2)
    bc_pool = open_pool("bc", 2)
    on_pool = open_pool("on", 2)
    dr_pool = open_pool("dr", 2, space="DRAM")
    for h in range(HPC):
        pt, hr = divmod(h, 2)
        rb = hr * 64
        ob_list = []
        for qg in range(NTG):
            qb = qg * TG
            qs = slice(qb, qb + TG)
            if qg == 0:
                kts = [(kt, None) for kt in range(4)]
            else:
                kts = ([(kt, None) for kt in range(qb // P)] +
                       [(qb // P + m, -m * P) for m in range(4)])
            po = ps_o.tile([D + 1, TG], f32, tag="o", name="po")
            nkt = len(kts)
            for i, (kt, off) in enumerate(kts):
                ks = slice(kt * P, (kt + 1) * P)
                pss = ps_s.tile([P, TG], f32, tag="s", name="pss")
                nc.tensor.matmul(pss[:], lhsT=kT_t[pt][rb:rb + 64, ks],
                                 rhs=qT_t[pt][rb:rb + 64, qs],
                                 start=True, stop=True)
                pT = pT_pool.tile([P, TG], sdt, tag="pT", name="pT")
                nc.scalar.activation(pT[:], pss[:], Exp, scale=0.125)
                if off is not None:
                    c0 = off + TG
                    nc.vector.tensor_mul(pT[:], pT[:], mask_t[:, c0:c0 + TG])
                nc.tensor.matmul(po[:], lhsT=v_t[kt][:, h, :], rhs=pT[:],
                                 start=(i == 0), stop=(i == nkt - 1))
            # Evacuate the whole accumulator at once (frees the PSUM bank
            # immediately so the next head's O matmuls never stall).
            ob = ob_pool.tile([D + 1, TG], f32, tag="ob", name="ob")
            ob_list.append(ob)
            nc.vector.tensor_copy(ob[:], po[:])
        # Reciprocal of all 4 denominator rows at once (partitions 0..3).
        dd = rc_pool.tile([NTG, TG], f32, tag="ddst", name="dd")
        for qg in range(NTG):
            nc.sync.dma_start(out=dd[qg:qg + 1, :],
                              in_=ob_list[qg][D:D + 1, :])
        rec = rc_pool.tile([NTG, TG], f32, tag="rec", name="rec")
        scr = rc_pool.tile([NTG, TG], f32, tag="scr", name="scr")
        nc.vector.reciprocal_approx_accurate(out=rec[:], in_=dd[:],
                                             scratch=scr[:])
        rec_d = dr_pool.tile([NTG, TG], f32, tag="rec_d", name="rec_d")
        nc.sync.dma_start(out=rec_d[:], in_=rec[:])
        for qg in range(NTG):
            row = rec_d[qg:qg + 1, :]
            bcast_in = bass.AP(tensor=row.tensor, offset=row.offset,
                               ap=[[0, D]] + [list(a) for a in row.ap[1:]])
            bc = bc_pool.tile([D, TG], f32, tag="bc", name="bc")
            nc.sync.dma_start(out=bc[:], in_=bcast_in)
            on = on_pool.tile([D, TG], sdt, tag="on", name="on")
            nc.vector.tensor_mul(on[:], ob_list[qg][0:D, :], bc[:])
            nc.sync.dma_start(out=yT_t[pt][rb:rb + 64, qg * TG:(qg + 1) * TG],
                              in_=on[:])
    close_pool(dr_pool)
    close_pool(on_pool)
    close_pool(bc_pool)
    close_pool(rc_pool)
    close_pool(ob_pool)
    close_pool(pT_pool)
    close_pool(ps_o)
    close_pool(ps_s)
    close_pool(v_pool)
    close_pool(qk_pool)

    # ---------------- Phase 3: out = y @ Wp (row-parallel partial) --------
    ps_p = open_pool("ps_p", 3, space="PSUM")
    wp_pool = open_pool("wp", 2 * NPAIR)
    ot_pool = open_pool("ot", 4)
    wp_t = {}
    for c in range(NPAIR):
        for ng in range(E // TG):
            t = wp_pool.tile([P, TG], sdt, tag="wp", name="wpc")
            nc.sync.dma_start(out=t[:], in_=src(wp[c * P:(c + 1) * P,
                                                   ng * TG:(ng + 1) * TG]))
            wp_t[(c, ng)] = t
    for tt in range(NTT):
        ts_ = slice(tt * P, (tt + 1) * P)
        for ng in range(E // TG):
            pp = ps_p.tile([P, TG], f32, tag="proj", name="pp")
            for c in range(NPAIR):
                nc.tensor.matmul(pp[:], lhsT=yT_t[c][:, ts_],
                                 rhs=wp_t[(c, ng)][:],
                                 start=(c == 0), stop=(c == NPAIR - 1))
            ot = ot_pool.tile([P, TG], f32, tag="ot", name="ot")
            nc.vector.tensor_copy(ot[:], pp[:])
            nc.sync.dma_start(out=out[ts_, ng * TG:(ng + 1) * TG], in_=ot[:])
    close_pool(ot_pool)
    close_pool(wp_pool)
    close_pool(ps_p)
    close_pool(yT_pool)
    close_pool(singles)


def _get_program(mode=None):
    mode = mode or MODE
    if mode not in _CACHE:
        _CACHE[mode] = _build_program(mode)
    return _CACHE[mode]


def make_in_maps(x, W_qkv, b_qkv, W_proj, mode=None):
    """Per-core input dicts: core c -> (batch c%4, head-group c//4)."""
    mode = mode or MODE
    x = np.asarray(x, np.float32)
    W_qkv = np.asarray(W_qkv, np.float32)
    b_qkv = np.asarray(b_qkv, np.float32)
    tri = ((np.arange(3 * TG)[None, :] - TG) >=
           np.arange(P)[:, None]).astype(np.float32)
    if mode == "bf16":
        import ml_dtypes
        cvt = lambda a: np.ascontiguousarray(a).astype(ml_dtypes.bfloat16)
    else:
        cvt = lambda a: np.ascontiguousarray(a, np.float32)
    in_maps = []
    for c in range(NCORES):
        b, g = c % B, c // B
        gs = slice(g * ESL, (g + 1) * ESL)
        in_maps.append({
            "xT": cvt(x[b].T),
            "wq": cvt(W_qkv[:, 0 * E:1 * E][:, gs]),
            "wk": cvt(W_qkv[:, 1 * E:2 * E][:, gs]),
            "wv": cvt(W_qkv[:, 2 * E:3 * E][:, gs]),
            "wp": cvt(np.asarray(W_proj, np.float32)[gs, :]),
            "bq": np.ascontiguousarray(b_qkv[0 * E:1 * E][gs, None]),
            "bk": np.ascontiguousarray(b_qkv[1 * E:2 * E][gs, None]),
            "trimask": cvt(tri),
        })
    return in_maps


def gather_output(results, b_qkv, b_proj, W_proj):
    """Sum the two row-parallel partials per batch; fold v/proj biases."""
    b_qkv = np.asarray(b_qkv, np.float64)
    W_proj = np.asarray(W_proj, np.float64)
    b_v = b_qkv[2 * E:3 * E]
    const = b_v @ W_proj + np.asarray(b_proj, np.float64)
    out = np.empty((B, T, E), np.float32)
    for b in range(B):
        out[b] = (results[b]["out"].astype(np.float64) +
                  results[b + B]["out"].astype(np.float64) +
                  const).astype(np.float32)
    return out


def run_on_hw(inputs, trace=False, mode=None, **kwargs):
    from concourse.bass_utils import run_bass_kernel_spmd
    mode = mode or MODE
    nc = _get_program(mode)
    in_maps = make_in_maps(inputs["x"], inputs["W_qkv"], inputs["b_qkv"],
                           inputs["W_proj"], mode=mode)
    res = run_bass_kernel_spmd(nc, in_maps, list(range(NCORES)), trace=trace,
                               **kwargs)
    out = gather_output(res.results, inputs["b_qkv"], inputs["b_proj"],
                        inputs["W_proj"])
    return out, res


def kernel(x, W_qkv, b_qkv, W_proj, b_proj):
    out, _ = run_on_hw({"x": x, "W_qkv": W_qkv, "b_qkv": b_qkv,
                        "W_proj": W_proj, "b_proj": b_proj})
    return out


# revision 24
# speedup vs baseline: 1.5752x; 1.3293x over previous
"""Trainium2 Bass kernel for nn_CausalSelfAttention_6442450944521.

Sparse-attention causal self-attention block:
  B=4, T=2048 (rows<512: full attention over cols<512; rows>=512: causal),
  E=1024, H=16, D=64.

Sharding: batch (4) x head-group (2 groups of 8 heads) across 8 cores.
Each core computes, for its (batch b, head-group g):
  qkv^T projections (Q^T,K^T in [D,T] layout; V in natural [T,D] layout),
  block-sparse attention via S^T = K Q^T tiles (softmax denominators come
  free from a ones-column packed next to V), and its row-slice of the
  output projection. The two head-group partials per batch are summed on
  the host (row-parallel tensor parallelism); v-bias and proj-bias are
  folded in exactly on the host.

Matmul inputs stream as bf16 (1 cyc/row on the PE) by default;
KMODE=f32r switches to float32r (2 cyc/row, ~20x lower error).
Accumulation is always fp32 in PSUM; softmax denominators and the
normalization are fp32.
"""

import os
import sys

if "/opt/trn_rl_repo" not in sys.path:
    sys.path.insert(0, "/opt/trn_rl_repo")

import numpy as np

# Problem constants (hardcoded per harness contract).
B = 4
T = 2048
E = 1024
H = 16
D = 64
NCORES = 8
HPC = H // 2          # heads per core = 8
ESL = HPC * D         # per-core E-slice = 512
P = 128               # SBUF/PSUM partitions
TG = 512              # matmul moving-dim tile (q-group width)
NTG = T // TG         # 4
NTT = T // P          # 16
NEC = E // P          # 8 contraction chunks over E
NPAIR = HPC // 2      # 4 head-pair tiles

MODE = os.environ.get("KMODE", "bf16")   # "bf16" | "f32r"

_CACHE = {}


def _build_program(mode):
    import concourse.bass as bass
    import concourse.tile as tile
    from concourse import bacc, mybir

    f32 = mybir.dt.float32
    idt = mybir.dt.bfloat16 if mode == "bf16" else mybir.dt.float32
    # dtype of DRAM inputs holding matmul operands (host converts for bf16)

    nc = bacc.Bacc("TRN2", target_bir_lowering=False, debug=False,
                   num_devices=NCORES)

    xT = nc.dram_tensor("xT", [E, T], idt, kind="ExternalInput").ap()
    wq = nc.dram_tensor("wq", [E, ESL], idt, kind="ExternalInput").ap()
    wk = nc.dram_tensor("wk", [E, ESL], idt, kind="ExternalInput").ap()
    wv = nc.dram_tensor("wv", [E, ESL], idt, kind="ExternalInput").ap()
    wp = nc.dram_tensor("wp", [ESL, E], idt, kind="ExternalInput").ap()
    bq = nc.dram_tensor("bq", [ESL, 1], f32, kind="ExternalInput").ap()
    bk = nc.dram_tensor("bk", [ESL, 1], f32, kind="ExternalInput").ap()
    trimask = nc.dram_tensor("trimask", [P, 3 * TG], idt,
                             kind="ExternalInput").ap()
    out = nc.dram_tensor("out", [T, E], f32, kind="ExternalOutput").ap()

    with tile.TileContext(nc) as tc:
        _body(nc, tc, tile, mybir, bass, mode,
              xT, wq, wk, wv, wp, bq, bk, trimask, out)

    nc.compile()
    return nc


def _body(nc, tc, tile, mybir, bass, mode,
          xT, wq, wk, wv, wp, bq, bk, trimask, out):
    f32 = mybir.dt.float32
    f32r = mybir.dt.float32r
    Exp = mybir.ActivationFunctionType.Exp
    Copy = mybir.ActivationFunctionType.Copy
    bf16_mode = mode == "bf16"
    # SBUF dtype for matmul operands; in f32r mode tiles are declared
    # float32r (walrus requires f32r-producing instructions) and DRAM
    # sources are bitcast.
    sdt = mybir.dt.bfloat16 if bf16_mode else f32r

    def src(ap):
        return ap if bf16_mode else ap.bitcast(f32r)

    cms = {}

    def open_pool(name, bufs, space=None, side=None):
        kw = {}
        if space:
            kw["space"] = space
        if side:
            kw["side"] = side
        cm = tc.tile_pool(name=name, bufs=bufs, **kw)
        pool = cm.__enter__()
        cms[id(pool)] = cm
        return pool

    def close_pool(pool):
        cms.pop(id(pool)).__exit__(None, None, None)

    def evac_bias(out_ap, psum_ap, bias_ap):
        if bf16_mode:
            nc.vector.tensor_scalar_add(out_ap, psum_ap, bias_ap)
        else:
            nc.scalar.add(out_ap, psum_ap, bias_ap)

    def evac_copy(out_ap, psum_ap):
        if bf16_mode:
            nc.vector.tensor_copy(out_ap, psum_ap)
        else:
            nc.scalar.activation(out_ap, psum_ap, Copy)

    singles = open_pool("singles", 1)          # mask + biases, whole program
    yT_pool = open_pool("yTpool", 1)           # y^T, until end of program

    mask_t = singles.tile([P, 3 * TG], sdt, tag="mask", name="mask")
    bias_t = singles.tile([P, 2 * NPAIR], f32, tag="bias", name="bias")
    nc.sync.dma_start(out=mask_t[:], in_=src(trimask))
    for pt in range(NPAIR):
        nc.sync.dma_start(out=bias_t[:, pt:pt + 1],
                          in_=bq[pt * P:(pt + 1) * P, :])
        nc.sync.dma_start(out=bias_t[:, NPAIR + pt:NPAIR + pt + 1],
                          in_=bk[pt * P:(pt + 1) * P, :])

    yT_t = [yT_pool.tile([P, T], sdt, tag=f"yT{i}", name=f"yT{i}")
            for i in range(NPAIR)]

    # ---------------- Phase 1a: Q^T, K^T = (x @ Wq|k)^T + bias ------------
    # Q^T is stored per head-PAIR [128, T]. K^T is stored per HEAD,
    # zero-padded to [128, T] (its 64 rows live at the head's position in
    # the pair; the other 64 rows are zero) so the S^T matmul loads a full
    # 128x128 stationary: half-utilization matmuls read as "idle" to the
    # PE activity monitor and throttle the clock to 1.2 GHz.
    qk_pool = open_pool("qkpool", 1, side="right")   # until end of attention
    qT_t = [qk_pool.tile([P, T], sdt, tag=f"qT{i}", name=f"qT{i}")
            for i in range(NPAIR)]
    kT_t = [qk_pool.tile([P, T], sdt, tag=f"kT{i}", name=f"kT{i}")
            for i in range(2 * NPAIR)]
    for hh in range(2 * NPAIR):
        zr = (1 - hh % 2) * 64    # zero the OTHER head's half
        nc.vector.memset(kT_t[hh][zr:zr + 64, :], 0.0)
    ps1 = open_pool("ps1", 4, space="PSUM")
    xs_pool = open_pool("xs", 2 * NEC)   # shared by QK strips and V strips
    wqk_pool = open_pool("wqk", 2 * NEC * NPAIR)
    wq_c, wk_c = {}, {}

    def load_wqk(pt):
        for ec in range(NEC):
            t = wqk_pool.tile([P, P], sdt, tag="wq", name="wqc")
            nc.sync.dma_start(out=t[:], in_=src(wq[ec * P:(ec + 1) * P,
                                                   pt * P:(pt + 1) * P]))
            wq_c[(pt, ec)] = t
            t = wqk_pool.tile([P, P], sdt, tag="wk", name="wkc")
            nc.sync.dma_start(out=t[:], in_=src(wk[ec * P:(ec + 1) * P,
                                                   pt * P:(pt + 1) * P]))
            wk_c[(pt, ec)] = t

    for tg in range(NTG):
        cs = slice(tg * TG, (tg + 1) * TG)
        xs_c = []
        for ec in range(NEC):
            t = xs_pool.tile([P, TG], sdt, tag="xs", name="xsc")
            nc.sync.dma_start(out=t[:], in_=src(xT[ec * P:(ec + 1) * P, cs]))
            xs_c.append(t)
        for pt in range(NPAIR):
            if tg == 0:
                load_wqk(pt)   # consumption order: overlap first matmuls
            psq = ps1.tile([P, TG], f32, tag="qk", name="psq")
            for ec in range(NEC):
                nc.tensor.matmul(psq[:], lhsT=wq_c[(pt, ec)][:],
                                 rhs=xs_c[ec][:],
                                 start=(ec == 0), stop=(ec == NEC - 1))
            evac_bias(qT_t[pt][:, cs], psq[:], bias_t[:, pt:pt + 1])
            psk = ps1.tile([P, TG], f32, tag="qk", name="psk")
            for ec in range(NEC):
                nc.tensor.matmul(psk[:], lhsT=wk_c[(pt, ec)][:],
                                 rhs=xs_c[ec][:],
                                 start=(ec == 0), stop=(ec == NEC - 1))
            evac_bias(kT_t[2 * pt][0:64, cs], psk[0:64, :],
                      bias_t[0:64, NPAIR + pt:NPAIR + pt + 1])
            evac_bias(kT_t[2 * pt + 1][64:P, cs], psk[64:P, :],
                      bias_t[64:P, NPAIR + pt:NPAIR + pt + 1])
    close_pool(wqk_pool)

    # ---------------- Phase 1b: V = x @ Wv (natural layout + ones col) ----
    # Per head: [V (64 cols) | ones | zeros (63)] -> full 128-col stationary
    # for the O matmul (same PE-activity-monitor reason as K^T padding).
    v_pool = open_pool("vpool", 1, side="right")     # until end of attention
    v_t = [v_pool.tile([P, HPC, P], sdt, tag=f"v{i}", name=f"v{i}")
           for i in range(NTT)]
    wv_pool = open_pool("wv", NEC)
    wv_c = []
    for ec in range(NEC):
        t = wv_pool.tile([P, ESL], sdt, tag="wv", name="wvc")
        nc.sync.dma_start(out=t[:], in_=src(wv[ec * P:(ec + 1) * P, :]))
        wv_c.append(t)
    for tt in range(NTT):
        ts_ = slice(tt * P, (tt + 1) * P)
        xv_c = []
        for ec in range(NEC):
            t = xs_pool.tile([P, P], sdt, tag="xv", name="xvc")
            nc.sync.dma_start(out=t[:], in_=src(xT[ec * P:(ec + 1) * P, ts_]))
            xv_c.append(t)
        psv = ps1.tile([P, HPC, D], f32, tag="qk", name="psv")
        for ec in range(NEC):
            nc.tensor.matmul(psv[:, :, :], lhsT=xv_c[ec][:],
                             rhs=wv_c[ec][:],
                             start=(ec == 0), stop=(ec == NEC - 1))
        zs = v_t[tt][:, :, D + 1:]
        nc.vector.memset(zs if bf16_mode else zs.bitcast(f32), 0.0)
        ones = v_t[tt][:, :, D:D + 1]
        nc.vector.memset(ones if bf16_mode else ones.bitcast(f32), 1.0)
        evac_copy(v_t[tt][:, :, 0:D], psv[:, :, :])
    close_pool(wv_pool)
    close_pool(xs_pool)
    close_pool(ps1)

    # ---------------- Phase 2: block-sparse attention ---------------------
    ps_s = open_pool("ps_s", 4, space="PSUM")
    ps_o = open_pool("ps_o", 4, space="PSUM")
    pT_pool = open_pool("pT", 6)
    ob_pool = open_pool("ob", 4)
    rc_pool = open_pool("rc", 2)
    bc_pool = open_pool("bc", 2)
    on_pool = open_pool("on", 2)
    dr_pool = open_pool("dr", 2, space="DRAM")
    for h in range(HPC):
        pt, hr = divmod(h, 2)
        rb = hr * 64
        ob_list = []
        for qg in range(NTG):
            qb = qg * TG
            qs = slice(qb, qb + TG)
            if qg == 0:
                kts = [(kt, None) for kt in range(4)]
            else:
                kts = ([(kt, None) for kt in range(qb // P)] +
                       [(qb // P + m, -m * P) for m in range(4)])
            po = ps_o.tile([P, TG], f32, tag="o", name="po")
            nkt = len(kts)
            for i, (kt, off) in enumerate(kts):
                ks = slice(kt * P, (kt + 1) * P)
                pss = ps_s.tile([P, TG], f32, tag="s", name="pss")
                nc.tensor.matmul(pss[:], lhsT=kT_t[h][:, ks],
                                 rhs=qT_t[pt][:, qs],
                                 start=True, stop=True)
                pT = pT_pool.tile([P, TG], sdt, tag="pT", name="pT")
                nc.scalar.activation(pT[:], pss[:], Exp, scale=0.125)
                if off is not None:
                    c0 = off + TG
                    nc.vector.tensor_mul(pT[:], pT[:], mask_t[:, c0:c0 + TG])
                nc.tensor.matmul(po[:], lhsT=v_t[kt][:, h, :], rhs=pT[:],
                                 start=(i == 0), stop=(i == nkt - 1))
            # Evacuate the whole accumulator at once (frees the PSUM bank
            # immediately so the next head's O matmuls never stall).
            ob = ob_pool.tile([D + 1, TG], f32, tag="ob", name="ob")
            ob_list.append(ob)
            nc.vector.tensor_copy(ob[:], po[0:D + 1, :])
        # Reciprocal of all 4 denominator rows at once (partitions 0..3).
        dd = rc_pool.tile([NTG, TG], f32, tag="ddst", name="dd")
        for qg in range(NTG):
            nc.sync.dma_start(out=dd[qg:qg + 1, :],
                              in_=ob_list[qg][D:D + 1, :])
        rec = rc_pool.tile([NTG, TG], f32, tag="rec", name="rec")
        scr = rc_pool.tile([NTG, TG], f32, tag="scr", name="scr")
        nc.vector.reciprocal_approx_accurate(out=rec[:], in_=dd[:],
                                             scratch=scr[:])
        rec_d = dr_pool.tile([NTG, TG], f32, tag="rec_d", name="rec_d")
        nc.sync.dma_start(out=rec_d[:], in_=rec[:])
        for qg in range(NTG):
            row = rec_d[qg:qg + 1, :]
            bcast_in = bass.AP(tensor=row.tensor, offset=row.offset,
                               ap=[[0, D]] + [list(a) for a in row.ap[1:]])
            bc = bc_pool.tile([D, TG], f32, tag="bc", name="bc")
            nc.sync.dma_start(out=bc[:], in_=bcast_in)
            on = on_pool.tile([D, TG], sdt, tag="on", name="on")
            nc.vector.tensor_mul(on[:], ob_list[qg][0:D, :], bc[:])
            nc.sync.dma_start(out=yT_t[pt][rb:rb + 64, qg * TG:(qg + 1) * TG],
                              in_=on[:])
    close_pool(dr_pool)
    close_pool(on_pool)
    close_pool(bc_pool)
    close_pool(rc_pool)
    close_pool(ob_pool)
    close_pool(pT_pool)
    close_pool(ps_o)
    close_pool(ps_s)
    close_pool(v_pool)
    close_pool(qk_pool)

    # ---------------- Phase 3: out = y @ Wp (row-parallel partial) --------
    ps_p = open_pool("ps_p", 3, space="PSUM")
    wp_pool = open_pool("wp", 2 * NPAIR)
    ot_pool = open_pool("ot", 4)
    wp_t = {}
    for c in range(NPAIR):
        for ng in range(E // TG):
            t = wp_pool.tile([P, TG], sdt, tag="wp", name="wpc")
            nc.sync.dma_start(out=t[:], in_=src(wp[c * P:(c + 1) * P,
                                                   ng * TG:(ng + 1) * TG]))
            wp_t[(c, ng)] = t
    for tt in range(NTT):
        ts_ = slice(tt * P, (tt + 1) * P)
        for ng in range(E // TG):
            pp = ps_p.tile([P, TG], f32, tag="proj", name="pp")
            for c in range(NPAIR):
                nc.tensor.matmul(pp[:], lhsT=yT_t[c][:, ts_],
                                 rhs=wp_t[(c, ng)][:],
                                 start=(c == 0), stop=(c == NPAIR - 1))
            ot = ot_pool.tile([P, TG], f32, tag="ot", name="ot")
            nc.vector.tensor_copy(ot[:], pp[:])
            nc.sync.dma_start(out=out[ts_, ng * TG:(ng + 1) * TG], in_=ot[:])
    close_pool(ot_pool)
    close_pool(wp_pool)
    close_pool(ps_p)
    close_pool(yT_pool)
    close_pool(singles)


def _get_program(mode=None):
    mode = mode or MODE
    if mode not in _CACHE:
        _CACHE[mode] = _build_program(mode)
    return _CACHE[mode]


def make_in_maps(x, W_qkv, b_qkv, W_proj, mode=None):
    """Per-core input dicts: core c -> (batch c%4, head-group c//4)."""
    mode = mode or MODE
    x = np.asarray(x, np.float32)
    W_qkv = np.asarray(W_qkv, np.float32)
    b_qkv = np.asarray(b_qkv, np.float32)
    tri = ((np.arange(3 * TG)[None, :] - TG) >=
           np.arange(P)[:, None]).astype(np.float32)
    if mode == "bf16":
        import ml_dtypes
        cvt = lambda a: np.ascontiguousarray(a).astype(ml_dtypes.bfloat16)
    else:
        cvt = lambda a: np.ascontiguousarray(a, np.float32)
    in_maps = []
    for c in range(NCORES):
        b, g = c % B, c // B
        gs = slice(g * ESL, (g + 1) * ESL)
        in_maps.append({
            "xT": cvt(x[b].T),
            "wq": cvt(W_qkv[:, 0 * E:1 * E][:, gs]),
            "wk": cvt(W_qkv[:, 1 * E:2 * E][:, gs]),
            "wv": cvt(W_qkv[:, 2 * E:3 * E][:, gs]),
            "wp": cvt(np.asarray(W_proj, np.float32)[gs, :]),
            "bq": np.ascontiguousarray(b_qkv[0 * E:1 * E][gs, None]),
            "bk": np.ascontiguousarray(b_qkv[1 * E:2 * E][gs, None]),
            "trimask": cvt(tri),
        })
    return in_maps


def gather_output(results, b_qkv, b_proj, W_proj):
    """Sum the two row-parallel partials per batch; fold v/proj biases."""
    b_qkv = np.asarray(b_qkv, np.float64)
    W_proj = np.asarray(W_proj, np.float64)
    b_v = b_qkv[2 * E:3 * E]
    const = b_v @ W_proj + np.asarray(b_proj, np.float64)
    out = np.empty((B, T, E), np.float32)
    for b in range(B):
        out[b] = (results[b]["out"].astype(np.float64) +
                  results[b + B]["out"].astype(np.float64) +
                  const).astype(np.float32)
    return out


def run_on_hw(inputs, trace=False, mode=None, **kwargs):
    from concourse.bass_utils import run_bass_kernel_spmd
    mode = mode or MODE
    nc = _get_program(mode)
    in_maps = make_in_maps(inputs["x"], inputs["W_qkv"], inputs["b_qkv"],
                           inputs["W_proj"], mode=mode)
    res = run_bass_kernel_spmd(nc, in_maps, list(range(NCORES)), trace=trace,
                               **kwargs)
    out = gather_output(res.results, inputs["b_qkv"], inputs["b_proj"],
                        inputs["W_proj"])
    return out, res


def kernel(x, W_qkv, b_qkv, W_proj, b_proj):
    out, _ = run_on_hw({"x": x, "W_qkv": W_qkv, "b_qkv": b_qkv,
                        "W_proj": W_proj, "b_proj": b_proj})
    return out


# revision 26
# speedup vs baseline: 2.1653x; 1.3746x over previous
"""Trainium2 Bass kernel for nn_CausalSelfAttention_6442450944521.

Sparse-attention causal self-attention block:
  B=4, T=2048 (rows<512: full attention over cols<512; rows>=512: causal),
  E=1024, H=16, D=64.

Sharding: batch (4) x head-group (2 groups of 8 heads) across 8 cores.
Each core computes, for its (batch b, head-group g):
  qkv^T projections (Q^T,K^T in [D,T] layout; V in natural [T,D] layout),
  block-sparse attention via S^T = K Q^T tiles (softmax denominators come
  free from a ones-column packed next to V), and its row-slice of the
  output projection. The two head-group partials per batch are summed on
  the host (row-parallel tensor parallelism); v-bias and proj-bias are
  folded in exactly on the host.

Matmul operands are bf16 (1 cyc/row on the PE); accumulation is fp32 in
PSUM; softmax denominators and normalization are fp32. All stationary
operands are zero-padded to full 128x128 — half-utilization matmuls read
as "idle" to the PE activity monitor, which then throttles the PE clock
to 1.2 GHz. V is computed first, then per head-pair Q/K immediately
followed by that pair's two attention heads, so the scheduler overlaps
the next pair's projections with attention.
"""

import os
import sys

if "/opt/trn_rl_repo" not in sys.path:
    sys.path.insert(0, "/opt/trn_rl_repo")

import numpy as np

# Problem constants (hardcoded per harness contract).
B = 4
T = 2048
E = 1024
H = 16
D = 64
NCORES = 8
HPC = H // 2          # heads per core = 8
ESL = HPC * D         # per-core E-slice = 512
P = 128               # SBUF/PSUM partitions
TG = 512              # matmul moving-dim tile (q-group width)
NTG = T // TG         # 4
NTT = T // P          # 16
NEC = E // P          # 8 contraction chunks over E
NPAIR = HPC // 2      # 4 head-pair tiles

_CACHE = {}


def _build_program():
    import concourse.bass as bass
    import concourse.tile as tile
    from concourse import bacc, mybir

    f32 = mybir.dt.float32
    bf16 = mybir.dt.bfloat16

    nc = bacc.Bacc("TRN2", target_bir_lowering=False, debug=False,
                   num_devices=NCORES)

    xT = nc.dram_tensor("xT", [E, T], bf16, kind="ExternalInput").ap()
    wq = nc.dram_tensor("wq", [E, ESL], bf16, kind="ExternalInput").ap()
    wk = nc.dram_tensor("wk", [E, ESL], bf16, kind="ExternalInput").ap()
    wv = nc.dram_tensor("wv", [E, ESL], bf16, kind="ExternalInput").ap()
    wp = nc.dram_tensor("wp", [ESL, E], bf16, kind="ExternalInput").ap()
    bq = nc.dram_tensor("bq", [ESL, 1], f32, kind="ExternalInput").ap()
    bk = nc.dram_tensor("bk", [ESL, 1], f32, kind="ExternalInput").ap()
    trimask = nc.dram_tensor("trimask", [P, 3 * TG], bf16,
                             kind="ExternalInput").ap()
    out = nc.dram_tensor("out", [T, E], f32, kind="ExternalOutput").ap()

    with tile.TileContext(nc) as tc:
        _body(nc, tc, tile, mybir, bass,
              xT, wq, wk, wv, wp, bq, bk, trimask, out)

    nc.compile()
    return nc


def _body(nc, tc, tile, mybir, bass,
          xT, wq, wk, wv, wp, bq, bk, trimask, out):
    f32 = mybir.dt.float32
    bf16 = mybir.dt.bfloat16
    Exp = mybir.ActivationFunctionType.Exp

    cms = {}

    def open_pool(name, bufs, space=None, side=None):
        kw = {}
        if space:
            kw["space"] = space
        if side:
            kw["side"] = side
        cm = tc.tile_pool(name=name, bufs=bufs, **kw)
        pool = cm.__enter__()
        cms[id(pool)] = cm
        return pool

    def close_pool(pool):
        cms.pop(id(pool)).__exit__(None, None, None)

    # ---- pools ----------------------------------------------------------
    singles = open_pool("singles", 1)
    yT_pool = open_pool("yTpool", 1)
    ps_all = open_pool("ps", 2, space="PSUM")        # per-tile bufs override
    xr_pool = open_pool("xr", 1)                     # resident x^T (bf16)
    w_pool = open_pool("w", 1)                       # resident weights
    pT_pool = open_pool("pT", 6)
    ob_pool = open_pool("ob", 5)
    rc_pool = open_pool("rc", 2)
    bc_pool = open_pool("bc", 2)
    on_pool = open_pool("on", 2)
    dr_pool = open_pool("dr", 2, space="DRAM")
    # right-stack: big attention-phase tensors
    qk_pool = open_pool("qkpool", 1, side="right")
    v_pool = open_pool("vpool", 1, side="right")

    # ---- resident tensors ------------------------------------------------
    mask_t = singles.tile([P, 3 * TG], bf16, tag="mask", name="mask")
    bias_t = singles.tile([P, 2 * NPAIR], f32, tag="bias", name="bias")
    nc.sync.dma_start(out=mask_t[:], in_=trimask)
    for pt in range(NPAIR):
        nc.sync.dma_start(out=bias_t[:, pt:pt + 1],
                          in_=bq[pt * P:(pt + 1) * P, :])
        nc.sync.dma_start(out=bias_t[:, NPAIR + pt:NPAIR + pt + 1],
                          in_=bk[pt * P:(pt + 1) * P, :])

    xr = []
    for ec in range(NEC):
        t = xr_pool.tile([P, T], bf16, tag=f"xr{ec}", name=f"xr{ec}")
        nc.sync.dma_start(out=t[:], in_=xT[ec * P:(ec + 1) * P, :])
        xr.append(t)

    yT_t = [yT_pool.tile([P, T], bf16, tag=f"yT{i}", name=f"yT{i}")
            for i in range(NPAIR)]
    qT_t = [qk_pool.tile([P, T], bf16, tag=f"qT{i}", name=f"qT{i}")
            for i in range(NPAIR)]
    # K^T per head, zero-padded to [128, T] (full-width PE stationary).
    kT_t = [qk_pool.tile([P, T], bf16, tag=f"kT{i}", name=f"kT{i}")
            for i in range(HPC)]
    for hh in range(HPC):
        zr = (1 - hh % 2) * 64
        nc.vector.memset(kT_t[hh][zr:zr + 64, :], 0.0)
    # V per T-tile: per head [V(64) | ones | zeros(63)] = 128-col stationary.
    v_t = [v_pool.tile([P, HPC, P], bf16, tag=f"v{i}", name=f"v{i}")
           for i in range(NTT)]

    wv_c, wq_c, wk_c, wp_c = [], {}, {}, {}
    for ec in range(NEC):
        t = w_pool.tile([P, ESL], bf16, tag="wv", name="wvc", bufs=NEC)
        nc.sync.dma_start(out=t[:], in_=wv[ec * P:(ec + 1) * P, :])
        wv_c.append(t)

    # ---- V = x @ Wv ------------------------------------------------------
    for tt in range(NTT):
        ts_ = slice(tt * P, (tt + 1) * P)
        psv = ps_all.tile([P, HPC, D], f32, tag="qk", name="psv", bufs=2)
        for ec in range(NEC):
            nc.tensor.matmul(psv[:, :, :], lhsT=xr[ec][:, ts_],
                             rhs=wv_c[ec][:],
                             start=(ec == 0), stop=(ec == NEC - 1))
        nc.vector.memset(v_t[tt][:, :, D + 1:], 0.0)
        nc.vector.memset(v_t[tt][:, :, D:D + 1], 1.0)
        nc.vector.tensor_copy(v_t[tt][:, :, 0:D], psv[:, :, :])

    # ---- per pair: Q/K projections, then the pair's two heads ------------
    for pt in range(NPAIR):
        for ec in range(NEC):
            t = w_pool.tile([P, P], bf16, tag="wq", name="wqc", bufs=2 * NEC)
            nc.sync.dma_start(out=t[:], in_=wq[ec * P:(ec + 1) * P,
                                              pt * P:(pt + 1) * P])
            wq_c[(pt, ec)] = t
            t = w_pool.tile([P, P], bf16, tag="wk", name="wkc", bufs=2 * NEC)
            nc.sync.dma_start(out=t[:], in_=wk[ec * P:(ec + 1) * P,
                                              pt * P:(pt + 1) * P])
            wk_c[(pt, ec)] = t
        for tg in range(NTG):
            cs = slice(tg * TG, (tg + 1) * TG)
            psq = ps_all.tile([P, TG], f32, tag="qk", name="psq", bufs=2)
            for ec in range(NEC):
                nc.tensor.matmul(psq[:], lhsT=wq_c[(pt, ec)][:],
                                 rhs=xr[ec][:, cs],
                                 start=(ec == 0), stop=(ec == NEC - 1))
            nc.vector.tensor_scalar_add(qT_t[pt][:, cs], psq[:],
                                        bias_t[:, pt:pt + 1])
            psk = ps_all.tile([P, TG], f32, tag="qk", name="psk", bufs=2)
            for ec in range(NEC):
                nc.tensor.matmul(psk[:], lhsT=wk_c[(pt, ec)][:],
                                 rhs=xr[ec][:, cs],
                                 start=(ec == 0), stop=(ec == NEC - 1))
            nc.vector.tensor_scalar_add(
                kT_t[2 * pt][0:64, cs], psk[0:64, :],
                bias_t[0:64, NPAIR + pt:NPAIR + pt + 1])
            nc.vector.tensor_scalar_add(
                kT_t[2 * pt + 1][64:P, cs], psk[64:P, :],
                bias_t[64:P, NPAIR + pt:NPAIR + pt + 1])

        # ---- attention for heads 2pt, 2pt+1 ----
        for h in (2 * pt, 2 * pt + 1):
            ob_list = []
            for qg in range(NTG):
                qb = qg * TG
                qs = slice(qb, qb + TG)
                if qg == 0:
                    kts = [(kt, None) for kt in range(4)]
                else:
                    kts = ([(kt, None) for kt in range(qb // P)] +
                           [(qb // P + m, -m * P) for m in range(4)])
                po = ps_all.tile([P, TG], f32, tag="o", name="po", bufs=3)
                nkt = len(kts)
                for i, (kt, off) in enumerate(kts):
                    ks = slice(kt * P, (kt + 1) * P)
                    pss = ps_all.tile([P, TG], f32, tag="s", name="pss",
                                      bufs=3)
                    nc.tensor.matmul(pss[:], lhsT=kT_t[h][:, ks],
                                     rhs=qT_t[pt][:, qs],
                                     start=True, stop=True)
                    pT = pT_pool.tile([P, TG], bf16, tag="pT", name="pT")
                    nc.scalar.activation(pT[:], pss[:], Exp, scale=0.125)
                    if off is not None:
                        c0 = off + TG
                        nc.vector.tensor_mul(pT[:], pT[:],
                                             mask_t[:, c0:c0 + TG])
                    nc.tensor.matmul(po[:], lhsT=v_t[kt][:, h, :], rhs=pT[:],
                                     start=(i == 0), stop=(i == nkt - 1))
                # Evacuate rows 0..64 (O and denominator); frees the bank.
                ob = ob_pool.tile([D + 1, TG], f32, tag="ob", name="ob")
                ob_list.append(ob)
                nc.vector.tensor_copy(ob[:], po[0:D + 1, :])
            # Reciprocal of all 4 denominator rows at once.
            dd = rc_pool.tile([NTG, TG], f32, tag="ddst", name="dd")
            for qg in range(NTG):
                nc.sync.dma_start(out=dd[qg:qg + 1, :],
                                  in_=ob_list[qg][D:D + 1, :])
            rec = rc_pool.tile([NTG, TG], f32, tag="rec", name="rec")
            scr = rc_pool.tile([NTG, TG], f32, tag="scr", name="scr")
            nc.vector.reciprocal_approx_accurate(out=rec[:], in_=dd[:],
                                                 scratch=scr[:])
            rec_d = dr_pool.tile([NTG, TG], f32, tag="rec_d", name="rec_d")
            nc.sync.dma_start(out=rec_d[:], in_=rec[:])
            rb = (h % 2) * 64
            for qg in range(NTG):
                row = rec_d[qg:qg + 1, :]
                bcast_in = bass.AP(tensor=row.tensor, offset=row.offset,
                                   ap=[[0, D]] + [list(a) for a in row.ap[1:]])
                bc = bc_pool.tile([D, TG], f32, tag="bc", name="bc")
                nc.sync.dma_start(out=bc[:], in_=bcast_in)
                on = on_pool.tile([D, TG], bf16, tag="on", name="on")
                nc.vector.tensor_mul(on[:], ob_list[qg][0:D, :], bc[:])
                nc.sync.dma_start(
                    out=yT_t[pt][rb:rb + 64, qg * TG:(qg + 1) * TG],
                    in_=on[:])

    # ---- proj: out = y @ Wp (row-parallel partial) -----------------------
    ot_pool = open_pool("ot", 4)
    for c in range(NPAIR):
        for ng in range(E // TG):
            t = w_pool.tile([P, TG], bf16, tag="wp", name="wpc", bufs=2 * NPAIR)
            nc.sync.dma_start(out=t[:], in_=wp[c * P:(c + 1) * P,
                                              ng * TG:(ng + 1) * TG])
            wp_c[(c, ng)] = t
    for tt in range(NTT):
        ts_ = slice(tt * P, (tt + 1) * P)
        for ng in range(E // TG):
            pp = ps_all.tile([P, TG], f32, tag="o", name="pp", bufs=3)
            for c in range(NPAIR):
                nc.tensor.matmul(pp[:], lhsT=yT_t[c][:, ts_],
                                 rhs=wp_c[(c, ng)][:],
                                 start=(c == 0), stop=(c == NPAIR - 1))
            ot = ot_pool.tile([P, TG], f32, tag="ot", name="ot")
            nc.vector.tensor_copy(ot[:], pp[:])
            nc.sync.dma_start(out=out[ts_, ng * TG:(ng + 1) * TG], in_=ot[:])

    close_pool(ot_pool)
    close_pool(v_pool)
    close_pool(qk_pool)
    close_pool(dr_pool)
    close_pool(on_pool)
    close_pool(bc_pool)
    close_pool(rc_pool)
    close_pool(ob_pool)
    close_pool(pT_pool)
    close_pool(w_pool)
    close_pool(xr_pool)
    close_pool(ps_all)
    close_pool(yT_pool)
    close_pool(singles)


def _get_program():
    if "nc" not in _CACHE:
        _CACHE["nc"] = _build_program()
    return _CACHE["nc"]


def make_in_maps(x, W_qkv, b_qkv, W_proj):
    """Per-core input dicts: core c -> (batch c%4, head-group c//4)."""
    import ml_dtypes
    x = np.asarray(x, np.float32)
    W_qkv = np.asarray(W_qkv, np.float32)
    b_qkv = np.asarray(b_qkv, np.float32)
    tri = ((np.arange(3 * TG)[None, :] - TG) >=
           np.arange(P)[:, None]).astype(np.float32)
    cvt = lambda a: np.ascontiguousarray(a).astype(ml_dtypes.bfloat16)
    in_maps = []
    for c in range(NCORES):
        b, g = c % B, c // B
        gs = slice(g * ESL, (g + 1) * ESL)
        in_maps.append({
            "xT": cvt(x[b].T),
            "wq": cvt(W_qkv[:, 0 * E:1 * E][:, gs]),
            "wk": cvt(W_qkv[:, 1 * E:2 * E][:, gs]),
            "wv": cvt(W_qkv[:, 2 * E:3 * E][:, gs]),
            "wp": cvt(np.asarray(W_proj, np.float32)[gs, :]),
            "bq": np.ascontiguousarray(b_qkv[0 * E:1 * E][gs, None]),
            "bk": np.ascontiguousarray(b_qkv[1 * E:2 * E][gs, None]),
            "trimask": cvt(tri),
        })
    return in_maps


def gather_output(results, b_qkv, b_proj, W_proj):
    """Sum the two row-parallel partials per batch; fold v/proj biases."""
    b_qkv = np.asarray(b_qkv, np.float64)
    W_proj = np.asarray(W_proj, np.float64)
    b_v = b_qkv[2 * E:3 * E]
    const = b_v @ W_proj + np.asarray(b_proj, np.float64)
    out = np.empty((B, T, E), np.float32)
    for b in range(B):
        out[b] = (results[b]["out"].astype(np.float64) +
                  results[b + B]["out"].astype(np.float64) +
                  const).astype(np.float32)
    return out


def run_on_hw(inputs, trace=False, **kwargs):
    from concourse.bass_utils import run_bass_kernel_spmd
    nc = _get_program()
    in_maps = make_in_maps(inputs["x"], inputs["W_qkv"], inputs["b_qkv"],
                           inputs["W_proj"])
    res = run_bass_kernel_spmd(nc, in_maps, list(range(NCORES)), trace=trace,
                               **kwargs)
    out = gather_output(res.results, inputs["b_qkv"], inputs["b_proj"],
                        inputs["W_proj"])
    return out, res


def kernel(x, W_qkv, b_qkv, W_proj, b_proj):
    out, _ = run_on_hw({"x": x, "W_qkv": W_qkv, "b_qkv": b_qkv,
                        "W_proj": W_proj, "b_proj": b_proj})
    return out
